# revision 12
# baseline (speedup 1.0000x reference)
"""Trainium2 Bass kernel for nn_Attention_54030688584207.

Single-head attention block:
    h = LN(x^T) ; qkv = h @ W^T + b ; S = q k^T / sqrt(N) + position
    out = softmax(S) @ v, returned as [B, C, N].

Sharding: 8 cores = 4 batches x 2 query-halves. Each core receives its
batch's x rotated so that its own 1024 query tokens come first, computes
LN + full K/V for the batch (replicated within the pair, no collectives),
q for its half, then scores/softmax/PV for its 1024 query rows.

Device layouts (per core):
    x_sh  [C=1024, N=2048] f32   channels x tokens (token-rotated)
    w_t   [C=1024, 3C=3072] f32  W'^T (gamma/SCALE folded on host)
    bias  [3072] f32             b' (beta folded, q-part scaled)
    pos_t [N=2048, MY=1024] bf16 position^T (rows in local key order)
    out   [MY=1024, C=1024] f32  out[i, c]  (host transposes back)
"""

import os
import sys

for _p in ("/opt/trn_rl_repo",):
    if _p not in sys.path and os.path.isdir(_p):
        sys.path.insert(0, _p)

import numpy as np
import ml_dtypes

import concourse.bass as bass
import concourse.tile as tile
from concourse import bacc, mybir
from concourse.bass import ts, ds
from concourse.bass_utils import run_bass_kernel_spmd

FP = mybir.dt.float32
BF = mybir.dt.bfloat16
F32R = mybir.dt.float32r
AF = mybir.ActivationFunctionType
OP = mybir.AluOpType

B = 4
C = 1024
N = 2048
MY = 1024  # query rows per core
D3 = 3 * C
NCH = C // 128   # 8 channel chunks
NJT = N // 128   # 16 key tiles
NIB = MY // 128  # 8 query blocks
LN_EPS = 1e-5
SCALE = 1.0 / np.sqrt(N)

V_SLAB = 256  # c-out columns per v-projection weight slab
POS_DT = BF  # position tiles on device


def build_kernel():
    nc = bacc.Bacc("TRN2", target_bir_lowering=False, debug=False, num_devices=8)
    x_ext = nc.declare_dram_parameter("x_sh", [C, N], BF, isOutput=False)
    wt_ext = nc.declare_dram_parameter("w_t", [C, D3], BF, isOutput=False)
    b_ext = nc.declare_dram_parameter("bias", [D3], FP, isOutput=False)
    pos_ext = nc.declare_dram_parameter("pos_t", [N, MY], POS_DT, isOutput=False)
    out_ext = nc.declare_dram_parameter("out", [MY, C], FP, isOutput=True)

    x_r = x_ext.ap().rearrange("(a p) n -> p a n", p=128)      # [128, 8, N]
    wt_r = wt_ext.ap().rearrange("(a p) d -> p a d", p=128)    # [128, 8, D3]
    b_r = b_ext.ap().rearrange("(a p) -> p a", p=128)          # [128, 24]

    with tile.TileContext(nc) as tc:
        with (
            tc.tile_pool(name="res", bufs=1) as res,
            tc.tile_pool(name="statb", bufs=2) as statb,
            tc.tile_pool(name="wpool", bufs=2) as wpool,
            tc.tile_pool(name="pospool", bufs=2) as pospool,
            tc.tile_pool(name="scr", bufs=2) as scr,
            tc.tile_pool(name="rows", bufs=1) as rows,
            tc.tile_pool(name="small", bufs=1) as small,
        ):
            # ---- resident tiles ----
            xh = res.tile([128, NCH, N], BF, tag="big")       # x, then h in-place
            qs = res.tile([128, NCH, MY], BF, tag="qs")       # q^T  [c, i]
            ks = res.tile([128, NCH, N], BF, tag="ks")        # k^T  [c, j]
            vs = res.tile([128, NJT, C], BF, tag="vs")        # v    [j, c]

            ones_f = rows.tile([128, 1], FP, tag="ones_f")
            nc.vector.memset(ones_f[:], 1.0)
            ones_b = rows.tile([128, 1], BF, tag="ones_b")
            nc.vector.memset(ones_b[:], 1.0)

            bias_sb = rows.tile([128, 24], FP, tag="bias")
            nc.sync.dma_start(bias_sb[:], b_r)
            # v-bias broadcast row [1, C] -> [128, C]
            bvrow = statb.tile([1, C], FP, tag="statb", name="bvrow")
            nc.sync.dma_start(bvrow[:], b_ext.ap()[ds(2 * C, C)].rearrange("(o c) -> o c", o=1))
            bv_b = rows.tile([128, C], FP, tag="bvb")
            nc.gpsimd.partition_broadcast(bv_b[:], bvrow[:])

            eps_t = rows.tile([1, 1], FP, tag="eps")
            nc.vector.memset(eps_t[:], LN_EPS)

            # broadcast LN stats, filled per 512-token chunk below
            mu_b = statb.tile([128, N], FP, tag="statb", name="mu_b")
            rstd_b = statb.tile([128, N], FP, tag="statb", name="rstd_b")

            # ---- Phase A: load x + LN stats ----
            for c in range(NCH):
                nc.sync.dma_start(xh[:, c, :], x_r[:, c, :])

            with tc.tile_pool(name="psA", bufs=2, space="PSUM") as psA:
                for t in range(N // 512):
                    ps_s = psA.tile([1, 512], FP, tag="ps_s")
                    ps_q = psA.tile([1, 512], FP, tag="ps_q")
                    for c in range(NCH):
                        xsq = scr.tile([128, 512], BF, tag="scratch2")
                        nc.scalar.square(xsq[:], xh[:, c, ts(t, 512)])
                        nc.tensor.matmul(
                            ps_s[:], ones_b[:], xh[:, c, ts(t, 512)],
                            start=(c == 0), stop=(c == NCH - 1))
                        nc.tensor.matmul(
                            ps_q[:], ones_b[:], xsq[:],
                            start=(c == 0), stop=(c == NCH - 1))
                    # mu = s/C ; e2 = q/C ; var = e2 - mu^2
                    mu_c = small.tile([1, 512], FP, tag="mu_c", name=f"mu_c{t}")
                    nc.scalar.mul(mu_c[:], ps_s[:], 1.0 / C)
                    e2 = small.tile([1, 512], FP, tag="e2", name=f"e2_{t}")
                    nc.scalar.mul(e2[:], ps_q[:], 1.0 / C)
                    mu2 = small.tile([1, 512], FP, tag="mu2", name=f"mu2_{t}")
                    nc.vector.tensor_mul(mu2[:], mu_c[:], mu_c[:])
                    var = small.tile([1, 512], FP, tag="var", name=f"var_{t}")
                    nc.vector.tensor_sub(var[:], e2[:], mu2[:])
                    # rstd = exp(-0.5 * ln(var + eps))
                    lnv = small.tile([1, 512], FP, tag="lnv", name=f"lnv_{t}")
                    nc.scalar.activation(lnv[:], var[:], AF.Ln, bias=eps_t[:])
                    rstd_c = small.tile([1, 512], FP, tag="rstd_c", name=f"rstd_c{t}")
                    nc.scalar.activation(rstd_c[:], lnv[:], AF.Exp, scale=-0.5)
                    nc.gpsimd.partition_broadcast(mu_b[:, ts(t, 512)], mu_c[:])
                    nc.gpsimd.partition_broadcast(rstd_b[:, ts(t, 512)], rstd_c[:])

            # apply LN in place: h = (x - mu) * rstd
            for c in range(NCH):
                nc.vector.tensor_sub(xh[:, c, :], xh[:, c, :], mu_b[:])
                nc.vector.tensor_mul(xh[:, c, :], xh[:, c, :], rstd_b[:])

            # ---- Phase B: QKV projections ----
            with tc.tile_pool(name="psB", bufs=4, space="PSUM") as psB:
                # q^T (d-tiles 0..7) and k^T (d-tiles 8..15), weights stationary
                for dt in range(16):
                    w_tile = wpool.tile([128, NCH, 128], BF, tag="wqk")
                    nc.sync.dma_start(w_tile[:], wt_r[:, :, ts(dt, 128)])
                    ntok = (MY // 512) if dt < 8 else (N // 512)
                    pss = [psB.tile([128, 512], FP, tag="qkv", name=f"qkv_{dt}_{t}")
                           for t in range(ntok)]
                    for c in range(NCH):
                        for t in range(ntok):
                            nc.tensor.matmul(
                                pss[t][:], w_tile[:, c, :],
                                xh[:, c, ts(t, 512)],
                                start=(c == 0), stop=(c == NCH - 1))
                    for t in range(ntok):
                        if dt < 8:
                            nc.vector.tensor_scalar_add(
                                qs[:, dt, ts(t, 512)], pss[t][:], bias_sb[:, dt:dt + 1])
                        else:
                            nc.vector.tensor_scalar_add(
                                ks[:, dt - 8, ts(t, 512)], pss[t][:], bias_sb[:, dt:dt + 1])

                # v, activations stationary: out v[j, c] accumulated over channel chunks
                for sl in range(C // V_SLAB):
                    wv = statb.tile([128, NCH, V_SLAB], BF, tag="statb", name=f"wv{sl}")
                    nc.sync.dma_start(wv[:], wt_r[:, :, ds(2 * C + sl * V_SLAB, V_SLAB)])
                    for jt in range(NJT):
                        psv = psB.tile([128, V_SLAB], FP, tag="psv")
                        for c in range(NCH):
                            nc.tensor.matmul(
                                psv[:], xh[:, c, ts(jt, 128)], wv[:, c, :],
                                start=(c == 0), stop=(c == NCH - 1))
                        nc.vector.tensor_add(
                            vs[:, jt, ds(sl * V_SLAB, V_SLAB)], psv[:],
                            bv_b[:, ds(sl * V_SLAB, V_SLAB)])

            # ---- Phase C: S^T = k^T.T q^T + pos ; exp -> es (bf16) ----
            es = res.tile([128, NJT, MY], BF, tag="big")  # reuses xh slot
            with tc.tile_pool(name="psC", bufs=2, space="PSUM") as psC:
                for j in range(NJT):
                    pos_tile = pospool.tile([128, MY], POS_DT, tag="pos")
                    nc.sync.dma_start(pos_tile[:], pos_ext[ts(j, 128), :])
                    psS = psC.tile([128, MY], FP, tag="S")
                    for c in range(NCH):
                        for ih in range(MY // 512):
                            nc.tensor.matmul(
                                psS[:, ts(ih, 512)], ks[:, c, ts(j, 128)],
                                qs[:, c, ts(ih, 512)],
                                start=(c == 0), stop=(c == NCH - 1))
                    nc.vector.tensor_add(psS[:], psS[:], pos_tile[:])
                    nc.scalar.activation(es[:, j, :], psS[:], AF.Exp)

            # ---- Phase D: out[i, c] = (P^T)^T v / rowsum ----
            with tc.tile_pool(name="psD", bufs=2, space="PSUM") as psD:
                for i in range(NIB):
                    pso = psD.tile([128, C], FP, tag="O")
                    ps_sum = psD.tile([128, 1], FP, tag="sum")
                    for j in range(NJT):
                        lhsT = es[:, j, ts(i, 128)]
                        for cc in range(C // 512):
                            nc.tensor.matmul(
                                pso[:, ts(cc, 512)], lhsT, vs[:, j, ts(cc, 512)],
                                start=(j == 0), stop=(j == NJT - 1))
                        nc.tensor.matmul(
                            ps_sum[:], lhsT, ones_b[:],
                            start=(j == 0), stop=(j == NJT - 1))
                    recip = small.tile([128, 1], FP, tag="recip", name=f"recip{i}")
                    nc.vector.reciprocal(recip[:], ps_sum[:])
                    out_t = statb.tile([128, C], FP, tag="statb", name=f"out_t{i}")
                    nc.vector.tensor_scalar_mul(out_t[:], pso[:], recip[:])
                    nc.sync.dma_start(out_ext[ts(i, 128), :], out_t[:])

    nc.compile()
    return nc


_NC_CACHE = None


def _get_nc():
    global _NC_CACHE
    if _NC_CACHE is None:
        _NC_CACHE = build_kernel()
    return _NC_CACHE


def prep_in_maps(x, position, ln_gamma, ln_beta, W_qkv, b_qkv):
    """Host-side sharding / layout prep. Returns in_maps for 8 cores."""
    x = np.asarray(x, dtype=np.float32)
    position = np.asarray(position, dtype=np.float32)
    ln_gamma = np.asarray(ln_gamma, dtype=np.float32)
    ln_beta = np.asarray(ln_beta, dtype=np.float32)
    W_qkv = np.asarray(W_qkv, dtype=np.float32)
    b_qkv = np.asarray(b_qkv, dtype=np.float32)

    # Fold gamma into W columns, beta into bias; fold SCALE into q slice.
    Wp = W_qkv * ln_gamma[None, :]
    bp = b_qkv + W_qkv @ ln_beta
    Wp[:C] *= SCALE
    bp[:C] *= SCALE
    w_t = np.ascontiguousarray(Wp.T).astype(ml_dtypes.bfloat16)  # [C, 3C]

    in_maps = []
    for core in range(8):
        b, s = divmod(core, 2)
        if s == 0:
            x_sh = x[b]
            pos_rot = position
        else:
            x_sh = np.roll(x[b], -MY, axis=1)
            pos_rot = np.roll(position, -MY, axis=1)
        pos_t = np.ascontiguousarray(pos_rot[s * MY:(s + 1) * MY, :].T)  # [N, MY]
        if POS_DT == BF:
            pos_t = pos_t.astype(ml_dtypes.bfloat16)
        in_maps.append({
            "x_sh": np.ascontiguousarray(x_sh).astype(ml_dtypes.bfloat16),
            "w_t": w_t,
            "bias": bp,
            "pos_t": pos_t,
        })
    return in_maps


def kernel(x, position, ln_gamma, ln_beta, W_qkv, b_qkv):
    nc = _get_nc()
    in_maps = prep_in_maps(x, position, ln_gamma, ln_beta, W_qkv, b_qkv)
    res = run_bass_kernel_spmd(nc, in_maps, core_ids=list(range(8)))
    out = np.empty((B, C, N), dtype=np.float32)
    for core in range(8):
        b, s = divmod(core, 2)
        out[b, :, s * MY:(s + 1) * MY] = res.results[core]["out"].T
    return out
